# revision 2
# baseline (speedup 1.0000x reference)
"""Causal self-attention (B=4, T=2048, C=2048, H=16, rope) on 8 trn2 cores.

Sharding: core c handles batch b = c//2 and head-group g = c%2 (heads
g*8..g*8+7).  Per core:
  phase 1: qkv projection (f32r matmuls), rope on q/k, v transposed to
           [t, d] bf16; q/k/v round-trip through DRAM.
  phase 2: per head: scores^T = k_tile^T @ q (f32r, transposed layout so
           attn@V needs no transpose), exp on ACT (scale folded in) ->
           bf16, causal mask on diagonal tiles, attn@V with a ones-column
           appended to v so the softmax denominator falls out of the same
           matmul; normalize with per-partition reciprocal; transpose y
           back to [c, t] on PE.
  phase 3: pairwise AllGather of y halves (cores 2b <-> 2b+1).
  phase 4: out[t, f-half] = y^T @ W_proj^T (bf16), f-half per core.
Host: pre-transposes x / W slices, applies the rope-pair permutation to
q/k weight rows so rope pairs (2m, 2m+1) become (m, 64+m) and rope is
partition-block-aligned on device.
"""
import sys

sys.path.insert(0, "/opt/trn_rl_repo")

import numpy as np
import ml_dtypes

import concourse.bass as bass
import concourse.tile as tile
from concourse import bacc, mybir
from concourse import bass_utils

F32 = mybir.dt.float32
F32R = mybir.dt.float32r
BF16 = mybir.dt.bfloat16
AF = mybir.ActivationFunctionType
ALU = mybir.AluOpType
BF16NP = ml_dtypes.bfloat16

B, T, C = 4, 2048, 2048
NH, D = 16, 128
HL = 8             # heads per core
NCT = C // 128     # 16 c-tiles
NTT = T // 128     # 16 t-tiles
SCALE = 1.0 / np.sqrt(D)
RG = [[0, 1], [2, 3], [4, 5], [6, 7]]


def _build():
    nc = bacc.Bacc("TRN2", target_bir_lowering=False, debug=False, num_devices=8)
    xT = nc.dram_tensor("xT", [C, T], F32R, kind="ExternalInput").ap()
    Wall = nc.dram_tensor("Wall", [C, 3 * HL * D], F32R, kind="ExternalInput").ap()
    WpT = nc.dram_tensor("WpT", [C, C // 2], BF16, kind="ExternalInput").ap()
    cos2 = nc.dram_tensor("cos2", [128, T], F32, kind="ExternalInput").ap()
    sin2 = nc.dram_tensor("sin2", [128, T], F32, kind="ExternalInput").ap()
    sgn = nc.dram_tensor("sgn", [128, 1], F32, kind="ExternalInput").ap()
    mask4 = nc.dram_tensor("mask4", [128, 4, 512], BF16, kind="ExternalInput").ap()
    ident = nc.dram_tensor("ident", [128, 128], BF16, kind="ExternalInput").ap()
    out = nc.dram_tensor("out", [T, C // 2], F32, kind="ExternalOutput").ap()

    xT_t = xT.rearrange("(ct c) t -> c ct t", c=128)
    Wall_t = Wall.rearrange("(ct c) r -> c ct r", c=128)
    WpT_t = WpT.rearrange("(ct c) f -> c ct f", c=128)

    with tile.TileContext(nc) as tc:
        with tc.tile_pool(name="dram", bufs=1, space="DRAM") as dram, \
             tc.tile_pool(name="const", bufs=1) as cpool:
            qkT_d = dram.tile([16, 128, T], F32R)       # 0-7 q heads, 8-15 k heads
            v_d = dram.tile([HL, NTT, 128, 128], BF16)  # [h, tt, t, d]
            yg_in = dram.tile([HL * D, T], BF16)
            yg_out = dram.tile([C, T], BF16)

            id_sb = cpool.tile([128, 128], BF16)
            nc.sync.dma_start(id_sb[:], ident)
            m4_sb = cpool.tile([128, 4, 512], BF16)
            nc.sync.dma_start(m4_sb[:], mask4)
            sg_sb = cpool.tile([128, 1], F32)
            nc.sync.dma_start(sg_sb[:], sgn)

            # ---------------- phase 1: qkv + rope + v transpose ----------------
            with tc.tile_pool(name="p1cs", bufs=1) as cs_pool, \
                 tc.tile_pool(name="p1x", bufs=18) as x_pool, \
                 tc.tile_pool(name="p1w", bufs=3) as w_pool, \
                 tc.tile_pool(name="p1ab", bufs=3) as ab_pool, \
                 tc.tile_pool(name="p1qr", bufs=4) as qr_pool, \
                 tc.tile_pool(name="p1v", bufs=4) as vb_pool, \
                 tc.tile_pool(name="p1ps", bufs=4, space="PSUM") as ps_pool, \
                 tc.tile_pool(name="p1tp", bufs=2, space="PSUM") as tp_pool:
                c2_sb = cs_pool.tile([128, T], F32)
                nc.sync.dma_start(c2_sb[:], cos2)
                s2_sb = cs_pool.tile([128, T], F32)
                nc.sync.dma_start(s2_sb[:], sin2)

                for ch in range(2):          # t-chunks of 1024
                    xc = []
                    for ct in range(NCT):
                        xct = x_pool.tile([128, 1024], F32R, name="xct")
                        nc.sync.dma_start(
                            xct[:], xT_t[:, ct, ch * 1024:(ch + 1) * 1024])
                        xc.append(xct)
                    for rt in range(24):     # 8 q, 8 k, 8 v tiles
                        wt = w_pool.tile([128, NCT, 128], F32R, name="wt")
                        nc.sync.dma_start(
                            wt[:], Wall_t[:, :, rt * 128:(rt + 1) * 128])
                        for ts in range(2):  # 512-wide pieces
                            t0 = ch * 1024 + ts * 512
                            ps = ps_pool.tile([128, 512], F32, name="qkvps")
                            for ct in range(NCT):
                                nc.tensor.matmul(
                                    ps[:], wt[:, ct, :],
                                    xc[ct][:, ts * 512:(ts + 1) * 512],
                                    start=(ct == 0), stop=(ct == NCT - 1))
                            if rt < 16:
                                # rope: A = p*cos2; B = swap(p)*sin2;
                                # qr = B*sgn + A   (sgn = [-1]*64 + [+1]*64)
                                a_t = ab_pool.tile([128, 512], F32, name="a_t")
                                nc.vector.tensor_mul(
                                    a_t[:], ps[:], c2_sb[:, t0:t0 + 512])
                                b_t = ab_pool.tile([128, 512], F32, name="b_t")
                                nc.vector.tensor_mul(
                                    b_t[0:64, :], ps[64:128, :],
                                    s2_sb[0:64, t0:t0 + 512])
                                nc.vector.tensor_mul(
                                    b_t[64:128, :], ps[0:64, :],
                                    s2_sb[64:128, t0:t0 + 512])
                                qr = qr_pool.tile([128, 512], F32R, name="qr")
                                nc.vector.scalar_tensor_tensor(
                                    qr[:], b_t[:], sg_sb[:], a_t[:],
                                    op0=ALU.mult, op1=ALU.add)
                                nc.sync.dma_start(
                                    qkT_d[rt][:, t0:t0 + 512], qr[:])
                            else:
                                hv = rt - 16
                                vb = vb_pool.tile([128, 512], BF16, name="vb")
                                nc.scalar.copy(vb[:], ps[:])
                                for qq in range(4):
                                    tp = tp_pool.tile([128, 128], BF16, name="vtp")
                                    nc.tensor.transpose(
                                        tp[:], vb[:, qq * 128:(qq + 1) * 128],
                                        id_sb[:])
                                    vt = vb_pool.tile([128, 128], BF16, name="vt")
                                    nc.vector.tensor_copy(vt[:], tp[:])
                                    nc.sync.dma_start(
                                        v_d[hv, ch * 8 + ts * 4 + qq], vt[:])

            # ---------------- phase 2: attention per head ----------------
            with tc.tile_pool(name="p2y", bufs=1) as y_pool:
                yT_all = y_pool.tile([128, HL * NTT * 128], BF16)
                with tc.tile_pool(name="p2qk", bufs=4) as qk_pool, \
                     tc.tile_pool(name="p2va", bufs=2) as va_pool, \
                     tc.tile_pool(name="p2eb", bufs=10) as eb_pool, \
                     tc.tile_pool(name="p2yn", bufs=3) as yn_pool, \
                     tc.tile_pool(name="p2rc", bufs=3) as rc_pool, \
                     tc.tile_pool(name="p2sp", bufs=2, space="PSUM") as sp_pool, \
                     tc.tile_pool(name="p2yp", bufs=2, space="PSUM") as yp_pool, \
                     tc.tile_pool(name="p2tp", bufs=2, space="PSUM") as tp2_pool:
                    for h in range(HL):
                        qt_sb = qk_pool.tile([128, T], F32R, name="qt_sb")
                        nc.sync.dma_start(qt_sb[:], qkT_d[h])
                        kt_sb = qk_pool.tile([128, T], F32R, name="kt_sb")
                        nc.sync.dma_start(kt_sb[:], qkT_d[8 + h])
                        va = va_pool.tile([128, NTT, 129], BF16, name="va")
                        nc.sync.dma_start(
                            va[:, :, 0:128],
                            v_d[h].rearrange("tt t d -> t tt d"))
                        nc.vector.memset(va[:, :, 128:129], 1.0)

                        for Q in range(4):           # 512-wide qi chunks
                            ebs = []                 # exp tiles for kj pairs
                            for b2 in range(2 * Q + 2):
                                sp = sp_pool.tile([128, 2, 512], F32, name="sp")
                                for jj in range(2):
                                    j = 2 * b2 + jj
                                    nc.tensor.matmul(
                                        sp[:, jj, :],
                                        kt_sb[:, j * 128:(j + 1) * 128],
                                        qt_sb[:, Q * 512:(Q + 1) * 512],
                                        start=True, stop=True)
                                eb = eb_pool.tile([128, 2, 512], BF16, name="eb")
                                nc.scalar.activation(
                                    eb[:], sp[:], AF.Exp, scale=float(SCALE))
                                if b2 == 2 * Q:
                                    nc.vector.tensor_mul(
                                        eb[:], eb[:], m4_sb[:, 0:2, :])
                                elif b2 == 2 * Q + 1:
                                    nc.vector.tensor_mul(
                                        eb[:], eb[:], m4_sb[:, 2:4, :])
                                ebs.append(eb)
                            for ql in range(4):
                                qt_i = Q * 4 + ql    # global qi tile
                                yp = yp_pool.tile([128, 129], F32, name="yp")
                                for j in range(qt_i + 1):
                                    nc.tensor.matmul(
                                        yp[:],
                                        ebs[j // 2][:, j % 2,
                                                    ql * 128:(ql + 1) * 128],
                                        va[:, j, :],
                                        start=(j == 0), stop=(j == qt_i))
                                rc = rc_pool.tile([128, 1], F32, name="rc")
                                nc.vector.reciprocal(rc[:], yp[:, 128:129])
                                yn = yn_pool.tile([128, 128], BF16, name="yn")
                                nc.vector.tensor_scalar_mul(
                                    yn[:], yp[:, 0:128], rc[:])
                                tp = tp2_pool.tile([128, 128], BF16, name="ytp")
                                nc.tensor.transpose(tp[:], yn[:], id_sb[:])
                                nc.vector.tensor_copy(
                                    yT_all[:, (h * NTT + qt_i) * 128:
                                           (h * NTT + qt_i + 1) * 128], tp[:])

                # ---------------- phase 3: exchange y halves ----------------
                nc.sync.dma_start(
                    yg_in[:].rearrange("(h d) (tt t) -> d h tt t", d=128, t=128),
                    yT_all[:].rearrange("d (h tt t) -> d h tt t", tt=NTT, t=128))
            nc.gpsimd.collective_compute(
                "AllGather", ALU.bypass,
                ins=[yg_in[:].opt()], outs=[yg_out[:].opt()],
                replica_groups=RG)

            # ---------------- phase 4: output projection ----------------
            with tc.tile_pool(name="p4y", bufs=1) as yf_pool, \
                 tc.tile_pool(name="p4w", bufs=1) as wp_pool, \
                 tc.tile_pool(name="p4o", bufs=4) as o_pool, \
                 tc.tile_pool(name="p4ps", bufs=4, space="PSUM") as pp_pool:
                yf = yf_pool.tile([128, NCT, NTT, 128], BF16)
                nc.sync.dma_start(
                    yf[:], yg_out[:].rearrange(
                        "(ct c) (tt t) -> c ct tt t", c=128, t=128))
                wp = wp_pool.tile([128, NCT, C // 2], BF16)
                nc.sync.dma_start(wp[:], WpT_t)
                for tt in range(NTT):
                    for fc in range(2):
                        pp = pp_pool.tile([128, 512], F32, name="pp")
                        for ct in range(NCT):
                            nc.tensor.matmul(
                                pp[:], yf[:, ct, tt, :],
                                wp[:, ct, fc * 512:(fc + 1) * 512],
                                start=(ct == 0), stop=(ct == NCT - 1))
                        ob = o_pool.tile([128, 512], F32, name="ob")
                        nc.vector.tensor_copy(ob[:], pp[:])
                        nc.sync.dma_start(
                            out[tt * 128:(tt + 1) * 128,
                                fc * 512:(fc + 1) * 512], ob[:])
    nc.compile()
    return nc


_NC = None


def _get_nc():
    global _NC
    if _NC is None:
        _NC = _build()
    return _NC


def _rope_tables():
    inv_freq = (1.0 / (10000.0 ** (np.arange(0, D, 2, dtype=np.float32) / D)))
    t = np.arange(T, dtype=np.float32)
    freqs = np.outer(t, inv_freq).astype(np.float32)      # [T, 64]
    cos = np.cos(freqs).T                                 # [64, T]
    sin = np.sin(freqs).T
    cos2 = np.concatenate([cos, cos], 0).astype(np.float32)
    sin2 = np.concatenate([sin, sin], 0).astype(np.float32)
    return cos2, sin2


def make_in_maps(x, W_attn, W_proj):
    perm = np.concatenate([np.arange(0, D, 2), np.arange(1, D, 2)])
    cos2, sin2 = _rope_tables()
    sgn = np.concatenate([-np.ones((64, 1)), np.ones((64, 1))]).astype(np.float32)
    # mask4[p, jj, c] = 1 where qi >= kj on diagonal tiles, offset o = 128*jj
    p_i = np.arange(128)[:, None, None]
    jj_i = np.arange(4)[None, :, None]
    c_i = np.arange(512)[None, None, :]
    mask4 = (c_i >= p_i + 128 * jj_i).astype(BF16NP)
    ident = np.eye(128, dtype=BF16NP)

    in_maps = []
    for core in range(8):
        b, g = core // 2, core % 2
        rows = slice(g * HL * D, (g + 1) * HL * D)
        Wq = W_attn[0 * C:1 * C][rows].reshape(HL, D, C)[:, perm, :].reshape(HL * D, C)
        Wk = W_attn[1 * C:2 * C][rows].reshape(HL, D, C)[:, perm, :].reshape(HL * D, C)
        Wv = W_attn[2 * C:3 * C][rows]
        Wall = np.ascontiguousarray(np.concatenate([Wq, Wk, Wv], 0).T)
        in_maps.append({
            "xT": np.ascontiguousarray(x[b].T),
            "Wall": Wall,
            "WpT": np.ascontiguousarray(
                W_proj[g * (C // 2):(g + 1) * (C // 2), :].T).astype(BF16NP),
            "cos2": cos2, "sin2": sin2, "sgn": sgn,
            "mask4": mask4, "ident": ident,
        })
    return in_maps


def _assemble(results):
    out = np.empty((B, T, C), dtype=np.float32)
    for core in range(8):
        b, g = core // 2, core % 2
        out[b][:, g * (C // 2):(g + 1) * (C // 2)] = results[core]["out"]
    return out


def run(x, W_attn, W_proj, **spmd_kwargs):
    nc = _get_nc()
    in_maps = make_in_maps(np.asarray(x, dtype=np.float32),
                           np.asarray(W_attn, dtype=np.float32),
                           np.asarray(W_proj, dtype=np.float32))
    res = bass_utils.run_bass_kernel_spmd(
        nc, in_maps, core_ids=list(range(8)), **spmd_kwargs)
    return _assemble(res.results), res


def kernel(x, W_attn, W_proj):
    out, _ = run(x, W_attn, W_proj)
    return out
